# revision 1
# baseline (speedup 1.0000x reference)
"""CostVolumeLoss Trainium2 kernel.

Computes: min over 5x5 window of per-pixel channel-mean L1 diff between
pred and zero-padded shifted target, then global mean. Data-parallel over
the batch dim (N=8) across 8 NeuronCores; each core reduces its image to a
single partial sum, host combines.

Per-core layout: H split into 4 tiles of 128 partitions, W in the free dim,
channels blocked in the free dim. For each dy in [-2,2] the target tile is
DMA-loaded row-shifted (zero halos memset), so every (dy,dx) offset becomes
a pure free-dim slice. Per offset: one tensor_tensor subtract, one fused
abs+channel-sum group tensor_reduce, one running-min tensor_tensor. The
last offset fuses the min with the spatial row-sum via
scalar_tensor_tensor(accum_out=...).
"""

import contextlib
import sys

if "/opt/trn_rl_repo" not in sys.path:
    sys.path.insert(0, "/opt/trn_rl_repo")

import numpy as np

import concourse.bass as bass
import concourse.mybir as mybir
from concourse.tile import TileContext
from concourse.bass_utils import run_bass_kernel_spmd

F32 = mybir.dt.float32
BF16 = mybir.dt.bfloat16
Op = mybir.AluOpType

N, C, H, W = 8, 3, 512, 512
KER = 5
PAD = (KER - 1) // 2  # 2
WP = W + 2 * PAD      # 516
NT = H // 128         # 4 H-tiles per core
NCORES = 8


def _split_waits(nc, maxw=1):
    """walrus in this env rejects >1 sem wait per instruction: split extra
    waits onto preceding NoOps on the same engine."""
    import bass_rust

    n = 0
    for fn in nc.m.functions:
        for blk in fn.blocks:
            out = []
            changed = False
            for inst in blk.instructions:
                si = inst.sync_info
                if si is not None and si.on_wait is not None and len(si.on_wait) > maxw:
                    waits = list(si.on_wait)
                    head, tail = waits[:-maxw], waits[-maxw:]
                    for i in range(0, len(head), maxw):
                        n += 1
                        out.append(
                            bass_rust.InstNoOp(
                                name=f"WSPLIT-{n}",
                                engine=inst.engine,
                                sync_info=mybir.SyncInfo(
                                    on_wait=head[i : i + maxw], on_update=[]
                                ),
                            )
                        )
                    si.on_wait = tail
                    changed = True
                out.append(inst)
            if changed:
                blk.instructions = out
    return n


def _emit_tile_pass_bf16(nc, pools, pred, targ, total, t, variant):
    """bf16 compute path: fp32 staging tiles are cast to bf16 on the Scalar
    engine, the 5x5 pass runs fully contiguous on DVE in bf16 (2x mode), the
    spatial row-sum upcasts back to fp32. variant 'bf16act' additionally
    moves the abs ops to the Scalar engine."""
    tpool, ppool, dpool, cpool, rpool = pools
    h0 = t * 128
    abs_on_act = variant == "bf16act"

    pstage = ppool.tile([128, C * W], F32, tag="pstage")
    nc.sync.dma_start(
        out=pstage[:].rearrange("p (c w) -> p c w", c=C),
        in_=pred[:, h0 : h0 + 128, :].rearrange("c h w -> h c w"),
    )
    ptile = ppool.tile([128, C * W], BF16, tag="ptile")
    nc.scalar.copy(ptile[:], pstage[:])
    pview = ptile[:].rearrange("p (c w) -> p c w", c=C)

    tviews = {}
    for dy in range(-PAD, PAD + 1):
        tstage = tpool.tile([128, C * WP], F32, tag="tstage")
        tsv = tstage[:].rearrange("p (c w) -> p c w", c=C)
        r0 = h0 + dy
        lo = max(0, r0)
        hi = min(H, r0 + 128)
        if lo > r0 or hi < r0 + 128:
            nc.vector.memset(tstage[:, :], 0.0)
        else:
            for c in range(C):
                nc.vector.memset(tsv[:, c, 0:PAD], 0.0)
                nc.vector.memset(tsv[:, c, PAD + W : WP], 0.0)
        nc.sync.dma_start(
            out=tsv[lo - r0 : hi - r0, :, PAD : PAD + W],
            in_=targ[:, lo:hi, :].rearrange("c h w -> h c w"),
        )
        ttb = tpool.tile([128, C * WP], BF16, tag="ttb")
        nc.scalar.copy(ttb[:], tstage[:])
        tviews[dy] = ttb[:].rearrange("p (c w) -> p c w", c=C)

    runmin = rpool.tile([128, W], BF16, tag="runminb")
    rowsum = rpool.tile([128, 1], F32, tag="rowsum")

    offsets = [
        (dy, dx) for dy in range(-PAD, PAD + 1) for dx in range(-PAD, PAD + 1)
    ]
    noff = len(offsets)
    for oi, (dy, dx) in enumerate(offsets):
        ttv = tviews[dy]
        d = dpool.tile([128, C * W], BF16, tag="db")
        tslice = ttv[:, :, PAD + dx : PAD + dx + W]
        csum = cpool.tile([128, W], BF16, tag="csumb")
        dv = d[:].rearrange("p (c w) -> p c w", c=C)
        for c in range(C):
            nc.vector.tensor_tensor(
                dv[:, c, :], pview[:, c, :], tslice[:, c, :], Op.subtract
            )
            if abs_on_act:
                nc.scalar.activation(
                    dv[:, c, :], dv[:, c, :], mybir.ActivationFunctionType.Abs
                )
            else:
                nc.vector.scalar_tensor_tensor(
                    dv[:, c, :], dv[:, c, :], -1.0, dv[:, c, :], Op.mult, Op.max
                )
        nc.vector.tensor_tensor(csum[:], dv[:, 0, :], dv[:, 1, :], Op.add)
        nc.vector.tensor_tensor(csum[:], csum[:], dv[:, 2, :], Op.add)
        if oi == 0:
            nc.vector.tensor_copy(runmin[:], csum[:])
        else:
            nc.vector.tensor_tensor(runmin[:], runmin[:], csum[:], Op.min)
    # row-sum with fp32 accumulation
    nc.vector.tensor_reduce(
        rowsum[:], runmin[:], mybir.AxisListType.X, Op.add
    )
    nc.vector.tensor_tensor(total[:], total[:], rowsum[:], Op.add)


def _emit_tile_pass(nc, pools, pred, targ, total, t, variant):
    """One H-tile (128 rows) of one image: full 5x5 cost-volume min pass,
    accumulating the spatial row-sums into `total`."""
    if variant in ("bf16", "bf16act"):
        return _emit_tile_pass_bf16(nc, pools, pred, targ, total, t, variant)
    tpool, ppool, dpool, cpool, rpool = pools
    h0 = t * 128

    ptile = ppool.tile([128, C * W], F32, tag="ptile")
    nc.sync.dma_start(
        out=ptile[:].rearrange("p (c w) -> p c w", c=C),
        in_=pred[:, h0 : h0 + 128, :].rearrange("c h w -> h c w"),
    )
    pview = ptile[:].rearrange("p (c w) -> p c w", c=C)

    tviews = {}
    dy_range = range(-PAD, PAD + 1) if variant != "noload" else [0]
    for dy in dy_range:
        tt = tpool.tile([128, C * WP], F32, tag="tt")
        ttv = tt[:].rearrange("p (c w) -> p c w", c=C)
        r0 = h0 + dy
        lo = max(0, r0)
        hi = min(H, r0 + 128)
        if lo > r0 or hi < r0 + 128:
            # engine partition starts must be 0/32/64/96: just zero the
            # whole tile on boundary tiles
            nc.vector.memset(tt[:, :], 0.0)
        else:
            for c in range(C):
                nc.vector.memset(ttv[:, c, 0:PAD], 0.0)
                nc.vector.memset(ttv[:, c, PAD + W : WP], 0.0)
        nc.sync.dma_start(
            out=ttv[lo - r0 : hi - r0, :, PAD : PAD + W],
            in_=targ[:, lo:hi, :].rearrange("c h w -> h c w"),
        )
        tviews[dy] = ttv
    if variant == "noload":
        for dy in range(-PAD, PAD + 1):
            tviews[dy] = tviews[0]

    runmin = rpool.tile([128, W], F32, tag="runmin")
    rowsum = rpool.tile([128, 1], F32, tag="rowsum")

    offsets = [
        (dy, dx) for dy in range(-PAD, PAD + 1) for dx in range(-PAD, PAD + 1)
    ]
    noff = len(offsets)
    for oi, (dy, dx) in enumerate(offsets):
        ttv = tviews[dy]
        d = dpool.tile([128, C * W], F32, tag="d")
        tslice = ttv[:, :, PAD + dx : PAD + dx + W]

        if variant in ("contig9", "contig9act"):
            # fully contiguous per-channel ops
            csum = cpool.tile([128, W], F32, tag="csum")
            dv = d[:].rearrange("p (c w) -> p c w", c=C)
            for c in range(C):
                nc.vector.tensor_tensor(
                    dv[:, c, :], pview[:, c, :], tslice[:, c, :], Op.subtract
                )
                if variant == "contig9act":
                    # abs on the otherwise-idle Scalar engine
                    nc.scalar.activation(
                        dv[:, c, :], dv[:, c, :],
                        mybir.ActivationFunctionType.Abs,
                    )
                else:
                    nc.vector.scalar_tensor_tensor(
                        dv[:, c, :], dv[:, c, :], -1.0, dv[:, c, :],
                        Op.mult, Op.max,
                    )
            nc.vector.tensor_tensor(csum[:], dv[:, 0, :], dv[:, 1, :], Op.add)
            nc.vector.tensor_tensor(csum[:], csum[:], dv[:, 2, :], Op.add)
            if oi == 0:
                nc.vector.tensor_copy(runmin[:], csum[:])
            elif oi < noff - 1:
                nc.vector.tensor_tensor(runmin[:], runmin[:], csum[:], Op.min)
            else:
                scratch = cpool.tile([128, W], F32, tag="scratch")
                nc.vector.scalar_tensor_tensor(
                    scratch[:], csum[:], 1.0, runmin[:],
                    Op.mult, Op.min, accum_out=rowsum[:],
                )
            continue

        if variant == "blocked":
            # d channel-blocked (contiguous write), reduce strided
            dout = d[:].rearrange("p (c w) -> p c w", c=C)
            dred = d[:].rearrange("p (c w) -> p w c", c=C)
        else:
            # d interleaved (c fastest): contiguous stream for the reduce
            dout = d[:].rearrange("p (w c) -> p c w", c=C)
            dred = d[:].rearrange("p (w c) -> p w c", c=C)
        nc.vector.tensor_tensor(dout, pview, tslice, Op.subtract)
        if oi == 0:
            nc.vector.tensor_reduce(
                runmin[:], dred, mybir.AxisListType.X, Op.add,
                apply_absolute_value=True,
            )
        else:
            csum = cpool.tile([128, W], F32, tag="csum")
            nc.vector.tensor_reduce(
                csum[:], dred, mybir.AxisListType.X, Op.add,
                apply_absolute_value=True,
            )
            if oi < noff - 1:
                nc.vector.tensor_tensor(runmin[:], runmin[:], csum[:], Op.min)
            else:
                # fused final min + spatial row-sum
                scratch = cpool.tile([128, W], F32, tag="scratch")
                nc.vector.scalar_tensor_tensor(
                    scratch[:], csum[:], 1.0, runmin[:],
                    Op.mult, Op.min, accum_out=rowsum[:],
                )
    nc.vector.tensor_tensor(total[:], total[:], rowsum[:], Op.add)


_LAST_EST_NS = None


def _build(repeat=1, variant="base"):
    global _LAST_EST_NS
    nc = bass.Bass()
    pred = nc.declare_dram_parameter("pred", [C, H, W], F32, isOutput=False)
    targ = nc.declare_dram_parameter("target", [C, H, W], F32, isOutput=False)
    out = nc.declare_dram_parameter("out", [1, 1], F32, isOutput=True)

    with TileContext(nc) as tc:
        with (
            tc.tile_pool(name="tpool", bufs=10) as tpool,
            tc.tile_pool(name="ppool", bufs=2) as ppool,
            tc.tile_pool(name="dpool", bufs=3) as dpool,
            tc.tile_pool(name="cpool", bufs=3) as cpool,
            tc.tile_pool(name="rpool", bufs=2) as rpool,
            tc.tile_pool(name="spool", bufs=1) as spool,
        ):
            total = spool.tile([128, 1], F32)
            nc.vector.memset(total[:], 0.0)

            pools = (tpool, ppool, dpool, cpool, rpool)
            loop_ctx = (
                tc.For_i(0, repeat, 1) if repeat > 1 else contextlib.nullcontext()
            )
            with loop_ctx:
                for t in range(NT):
                    _emit_tile_pass(nc, pools, pred, targ, total, t, variant)

            red = spool.tile([1, 1], F32)
            nc.gpsimd.tensor_reduce(red[:], total[:], mybir.AxisListType.C, Op.add)
            nc.sync.dma_start(out=out[0:1, 0:1], in_=red[:])

    _LAST_EST_NS = (
        max(e[2] for e in tc._perfetto_entries) if tc._perfetto_entries else None
    )

    _split_waits(nc, 1)
    return nc


_NC_CACHE = None


def _get_nc():
    global _NC_CACHE
    if _NC_CACHE is None:
        # "contig9" measured fastest on HW: 152us/core-pass vs 487us for the
        # grouped-reduce layout (strided APs are heavily penalized) and vs
        # 299-400us for bf16 variants (casts cost more than 2x mode saves).
        _NC_CACHE = _build(variant="contig9")
    return _NC_CACHE


def kernel(pred, target_warpped, _trace=False):
    pred = np.ascontiguousarray(np.asarray(pred, dtype=np.float32))
    targ = np.ascontiguousarray(np.asarray(target_warpped, dtype=np.float32))
    assert pred.shape == (N, C, H, W) and targ.shape == (N, C, H, W)

    nc = _get_nc()
    in_maps = [
        {"pred": np.ascontiguousarray(pred[i]), "target": np.ascontiguousarray(targ[i])}
        for i in range(NCORES)
    ]
    res = run_bass_kernel_spmd(nc, in_maps, core_ids=list(range(NCORES)), trace=_trace)
    partials = np.array(
        [res.results[i]["out"][0, 0] for i in range(NCORES)], dtype=np.float64
    )
    loss = partials.sum() / (C * N * H * W)
    out = np.float32(loss)
    if _trace:
        return out, res
    return out

